# revision 17
# baseline (speedup 1.0000x reference)
"""Trainium2 Bass kernel for the FNO-style FourierLayer.

  x: [8, 512, 512, 32] f32 -> rfft2 over (h, w) -> keep 32x32 modes ->
  per-mode (C x C) channel mix with W[32, 32, 32, 32] -> zero-pad -> irfft2.

Strategy: data-parallel over batch, one sample per NeuronCore (8 cores).
Only 32 of 512 frequencies survive, so instead of an FFT each core runs a
chain of small dense real matmuls against DFT basis matrices (bf16 operands,
fp32 PSUM accumulation):

  A:   P = F^T X         contract h       -> P[kxri 64, (c w)]
  T1:  XBAR DMA transpose (scalar queue)  -> PT[w 128, (c kxri)]
  B:   raw = G^T PT      contract w       -> raw[ryky 64, (rx kx c)]
  Tc:  XBAR DMA transpose                 -> rawT[c 32, (rx kx ry ky)]
  CMB: complex re/im combine -> block-diagonal lhsT
       diag[32*kyl + c, kx*64 + kyg*8 + 2*kyl + ri] = low[kx, ky, ri, c]
       (split across GpSimd/Vector/Scalar)
  C:   per-(kx, ky-group-of-4) matmuls  OL = diag^T W  (256 matmuls of
       32 cols each; out rows (2*kyl+ri) at PSUM quadrant kyg%3)
  PRM: 3 accumulating permutation matmuls -> OLT[64 (2ky+ri), (kx d)]
  D:   u = Dab^T OLT      contract 2ky+ri -> u[w 128, (d ab kx)]
  T2:  XBAR DMA transpose                 -> uT[(ab kx) 64, (d w)]
  E:   out = Einv^T uT    contract (ab kx) -> [h 128, (d w)] -> DMA out

All inter-stage transposes run as single XBAR DMA-transpose instructions
on the Scalar HWDGE queue (14ns per 16x128 tile), leaving the PE free for
real matmuls; the bulk x/out traffic stays on the Sync queue so the
transposes are never queued behind it.  PSUM evacuation is batched as
[*, 1024] two-bank copies rotated across Vector/Scalar; evac copy APs
reorder columns so each DMA transpose directly produces the layout the
next matmul stage wants.  D/T2/E are software-pipelined per w-chunk so
output DMA starts as early as possible.

DFT matrices are built on host from np.fft basis responses (this captures
the irfft Im(DC)-drop convention exactly). x, W and the matrices are cast
to bf16 on host and the output is returned as bf16 and upcast on host,
which halves DMA traffic in both directions.
"""
import numpy as np
import ml_dtypes

import concourse.bass as bass
import concourse.bacc as bacc
import concourse.mybir as mybir
from concourse import tile
from concourse.bass_utils import run_bass_kernel_spmd

B, H, W_, C = 8, 512, 512, 32
MODES = 32
N = 512
NCORES = 8

BF = mybir.dt.bfloat16
F32 = mybir.dt.float32


def _make_consts():
    h = np.arange(N)
    k = np.arange(MODES)
    ang = 2 * np.pi * np.outer(h, k) / N
    F = np.concatenate([np.cos(ang), -np.sin(ang)], axis=1)      # [512, 64]

    eye = np.eye(MODES)
    zc = np.concatenate([eye, np.zeros((MODES, N // 2 + 1 - MODES))], axis=1)
    row_re = np.fft.irfft(zc, n=N, axis=1)                        # [32, 512]
    row_im = np.fft.irfft(1j * zc, n=N, axis=1)

    # rows in interleaved (2*ky + ri) order, matching OLT rows
    Da = np.zeros((64, N))
    Db = np.zeros((64, N))
    Da[0::2] = row_re
    Da[1::2] = row_im
    Db[0::2] = row_im
    Db[1::2] = -row_re

    Einv = np.concatenate([np.cos(ang).T, np.sin(ang).T], axis=0) / N  # [64, 512]

    # F_sb[p, k*64+j] = F[k*128+p, j]
    F_sb = F.reshape(4, 128, 64).transpose(1, 0, 2).reshape(128, 256)
    Dab_sb = np.concatenate([Da, Db], axis=1)                          # [64, 1024]
    ident = np.eye(128)

    # permutation lhsT assembling OLT rows from C-stage psum quadrants:
    # psum tile t holds ky-group kyg = 3t + q at partition rows 32q + r;
    # OLT row = 8*kyg + r. Pt[32q + r, 8*(3t+q) + r] = 1.
    perm = np.zeros((3, 128, 64))
    for kyg in range(8):
        t, q = divmod(kyg, 3)
        for r in range(8):
            perm[t, 32 * q + r, 8 * kyg + r] = 1.0
    return (F_sb.astype(ml_dtypes.bfloat16), Dab_sb.astype(ml_dtypes.bfloat16),
            Einv.astype(ml_dtypes.bfloat16), ident.astype(ml_dtypes.bfloat16),
            perm.astype(ml_dtypes.bfloat16))


def _build_nc():
    F_np, Dab_np, Einv_np, idb_np, perm_np = _make_consts()

    nc = bacc.Bacc()
    x_d = nc.dram_tensor("x", [H, W_ * C], BF, kind="ExternalInput")
    # wpe[kyl*32 + c, kx*256 + kyg*32 + d] = W[kx, kyg*4 + kyl, c, d]
    wpe_d = nc.dram_tensor("wpe", [128, 8192], BF, kind="ExternalInput")
    out_d = nc.dram_tensor("out", [H, W_ * C], BF, kind="ExternalOutput")

    f_c = nc.inline_tensor(F_np, name="f_const")
    dab_c = nc.inline_tensor(Dab_np, name="dab_const")
    einv_c = nc.inline_tensor(Einv_np, name="einv_const")
    perm_c = nc.inline_tensor(
        np.ascontiguousarray(perm_np.transpose(1, 0, 2).reshape(128, 192)),
        name="perm_const")

    with tile.TileContext(nc) as tc:
        with (
            tc.tile_pool(name="const", bufs=1) as cpool,
            tc.tile_pool(name="xp", bufs=8) as xpool,
            tc.tile_pool(name="mid", bufs=2) as midpool,
            tc.tile_pool(name="ptp", bufs=2) as ptpool,
            tc.tile_pool(name="wp", bufs=1) as wpool,
            tc.tile_pool(name="sml", bufs=1) as smlpool,
            tc.tile_pool(name="up", bufs=2) as upool,
            tc.tile_pool(name="utp", bufs=2) as utpool,
            tc.tile_pool(name="osb", bufs=3) as opool,
        ):
            # ---- constants ----
            F_sb = cpool.tile([128, 256], BF)
            Dab_sb = cpool.tile([64, 1024], BF)
            Einv_sb = cpool.tile([64, 512], BF)
            warm_sb = cpool.tile([128, 128], BF)
            perm_sb = cpool.tile([128, 192], BF)
            wpe_sb = wpool.tile([128, 8192], BF)

            # raw[ryky 64, (rx*32 + kx)*128 + c]; cols 32..127 of each
            # 128-block are padding so the XBAR transpose (which always
            # sends source col j to out partition j%128) lands c on
            # partitions 0..31.
            raw_sb = smlpool.tile([64, 8192], BF)
            # rawT[c 32 (+96 garbage rows), rx*2048 + kx*64 + ry*32 + ky]
            rawT = smlpool.tile([128, 4096], BF)
            diag = smlpool.tile([128, 2048], BF)
            OL2 = smlpool.tile([128, 3072], BF)
            OLT = smlpool.tile([64, 1024], BF)

            # warm tile via memset: no DMA dependency, PE can start ~t=0
            nc.gpsimd.memset(warm_sb[:], 0.25)
            nc.gpsimd.memset(diag[:], 0.0)

            # F first (needed by first A matmul), then x streams in.
            nc.sync.dma_start(F_sb[:], f_c[:])

            # PE warmup: open the p-state ramp while first x tiles fly
            with tc.tile_pool(name="ps_w", bufs=1,
                              space=bass.MemorySpace.PSUM) as psw:
                wps = psw.tile([128, 512], F32, tag="wps", name="wps")
                for wi in range(56):
                    nc.tensor.matmul(
                        wps[:, 0:128], warm_sb[:], warm_sb[:],
                        start=True, stop=True)

            # rotating PSUM->SBUF evacuation (GPSIMD cannot access PSUM,
            # so only DVE + ACT share this work)
            _rot = [nc.vector.tensor_copy, nc.scalar.copy]
            _rix = [0]

            def rcopy(dst, src):
                fn = _rot[_rix[0] % 2]
                _rix[0] += 1
                fn(dst, src)

            # CMB split: GpSimd is ~3x slower per element than DVE, so it
            # gets 2 of the 8 combine ops and DVE the other 6.
            _cmb = [nc.vector.tensor_tensor, nc.gpsimd.tensor_tensor,
                    nc.vector.tensor_tensor, nc.vector.tensor_tensor,
                    nc.vector.tensor_tensor, nc.gpsimd.tensor_tensor,
                    nc.vector.tensor_tensor, nc.vector.tensor_tensor]
            _cix = [0]

            def ccomb(dst, a, b, op):
                fn = _cmb[_cix[0] % 8]
                _cix[0] += 1
                fn(dst, a, b, op)

            with (
                tc.tile_pool(name="ps_acc", bufs=4,
                             space=bass.MemorySpace.PSUM) as psa,
                tc.tile_pool(name="ps_pa", bufs=2,
                             space=bass.MemorySpace.PSUM) as ppa,
            ):
                # persistent stage-B accumulators: pb2[t] half h = ns 2t+h
                pb2 = [psa.tile([64, 1024], F32, tag="pb", bufs=2,
                                name=f"pb{i}") for i in range(2)]

                # ============= A + T1 + B, software-pipelined ============
                def emit_A(wq):
                    # P[kxri 64, c*128 + w]  (c outer so the XBAR transpose
                    # lands as PT[w, (c kxri)])
                    P_wq = midpool.tile([64, 4096], BF, tag="mid",
                                        name=f"P{wq}")
                    P3 = P_wq.rearrange("p (c w) -> p w c", w=128)
                    xk = []
                    for k in range(4):
                        t = xpool.tile([128, 4096], BF, tag="xk",
                                       name=f"x{wq}{k}")
                        xk.append(t)
                    # half-tile DMA slices in consumption order: the first
                    # pa group only waits for the first 4 half-slices.
                    for half in range(2):
                        for k in range(4):
                            nc.sync.dma_start(
                                xk[k][:, half * 2048:(half + 1) * 2048],
                                x_d[k * 128:(k + 1) * 128,
                                    wq * 4096 + half * 2048:
                                    wq * 4096 + (half + 1) * 2048])
                    if wq == 1:
                        nc.sync.dma_start(Dab_sb[:], dab_c[:])
                        nc.sync.dma_start(Einv_sb[:], einv_c[:])
                        nc.sync.dma_start(perm_sb[:], perm_c[:])
                    for pr in range(4):
                        pa2 = ppa.tile([64, 1024], F32, tag="pa",
                                       name=f"pa{wq}{pr}")
                        for hf in range(2):
                            ns = pr * 2 + hf
                            for k in range(4):
                                nc.tensor.matmul(
                                    pa2[:, hf * 512:(hf + 1) * 512],
                                    F_sb[:, k * 64:(k + 1) * 64],
                                    xk[k][:, ns * 512:(ns + 1) * 512],
                                    start=(k == 0), stop=(k == 3))
                        # src cols (w32, c32) -> dest P[c*128 + w] slice,
                        # both enumerated (w, c)
                        src = pa2.rearrange("p (w c) -> p w c", c=32)
                        rcopy(P3[:, pr * 32:(pr + 1) * 32, :], src)
                    return P_wq

                def emit_T1B(wq, P_wq):
                    PT_wq = ptpool.tile([128, 2048], BF, tag="pt",
                                        name=f"PT{wq}")
                    # XBAR: out[w, c, kxri] = P[kxri, c*128 + w]
                    nc.scalar.dma_start_transpose(
                        PT_wq.rearrange("p (c k) -> p c k", c=32),
                        P_wq[:])
                    for t in range(2):
                        for hf in range(2):
                            ns = 2 * t + hf
                            nc.tensor.matmul(
                                pb2[t][:, hf * 512:(hf + 1) * 512],
                                F_sb[:, wq * 64:(wq + 1) * 64],
                                PT_wq[:, ns * 512:(ns + 1) * 512],
                                start=(wq == 0), stop=(wq == 3))

                P_prev = emit_A(0)
                for wq in range(1, 4):
                    P_cur = emit_A(wq)
                    emit_T1B(wq - 1, P_prev)
                    P_prev = P_cur
                # W arrives after all x: off the phase-in critical path,
                # well before stage C needs it.
                nc.sync.dma_start(wpe_sb[:], wpe_d[:])
                emit_T1B(3, P_prev)

                # pb cols (c16, rx, kx) -> raw[ryky, (rx*32+kx)*128 + c]
                rawv = raw_sb.rearrange("p (rx kx c) -> p rx kx c",
                                        rx=2, kx=32, c=128)
                for t in range(2):
                    src = pb2[t].rearrange("p (c rx kx) -> p rx kx c",
                                           c=16, rx=2)
                    rcopy(rawv[:, :, :, 16 * t:16 * (t + 1)], src)

            # ====== Tc (XBAR) + CMB -> diag; C =======
            # rawT[c, rx*2048 + kx*64 + ry*32 + kyg*4 + kyl]
            nc.scalar.dma_start_transpose(
                rawT.rearrange("p (m l) -> p m l", m=64), raw_sb[:])
            rTv = rawT.rearrange(
                "p (rx kx ry kyg kyl) -> p rx kx ry kyg kyl",
                rx=2, kx=32, ry=2, kyg=8)
            diag_v = diag.rearrange("p (kx kyg r) -> p kx kyg r",
                                    kx=32, kyg=8)
            for kyl in range(4):
                prow = slice(32 * kyl, 32 * kyl + 32)
                ccomb(diag_v[prow, :, :, 2 * kyl],
                      rTv[0:32, 0, :, 0, :, kyl],
                      rTv[0:32, 1, :, 1, :, kyl],
                      mybir.AluOpType.subtract)
                ccomb(diag_v[prow, :, :, 2 * kyl + 1],
                      rTv[0:32, 0, :, 1, :, kyl],
                      rTv[0:32, 1, :, 0, :, kyl],
                      mybir.AluOpType.add)

            with tc.tile_pool(name="ps_c", bufs=2,
                              space=bass.MemorySpace.PSUM) as pcp:
                # C: 256 matmuls, out rows (2*kyl+ri); PSUM out partition
                # base must be 0/32/64, so 3 ky-groups per psum tile.
                # Unused psum rows are memset to 0 (the permutation matmul
                # below multiplies them by 0, and 0*NaN would poison it).
                pC = [pcp.tile([128, 1024], F32, tag="pC", bufs=3,
                               name=f"pC{i}") for i in range(3)]
                for t in range(3):
                    nc.vector.memset(pC[t][:], 0.0)

                for kx in range(32):
                    for kyg in range(8):
                        t, q = divmod(kyg, 3)
                        nc.tensor.matmul(
                            pC[t][32 * q:32 * q + 8,
                                  kx * 32:(kx + 1) * 32],
                            diag[:, kx * 64 + kyg * 8:kx * 64 + kyg * 8 + 8],
                            wpe_sb[:, kx * 256 + kyg * 32:
                                   kx * 256 + kyg * 32 + 32],
                            start=True, stop=True)
                for t in range(3):
                    rcopy(OL2[:, t * 1024:(t + 1) * 1024], pC[t][:])

            # assemble OLT rows (8*kyg + r) from the psum quadrant layout
            # with 3 accumulating permutation matmuls (disjoint out rows)
            with tc.tile_pool(name="ps_pm", bufs=1,
                              space=bass.MemorySpace.PSUM) as ppm:
                pm = ppm.tile([64, 1024], F32, tag="pm", name="pm")
                for h in range(2):
                    for t in range(3):
                        nc.tensor.matmul(
                            pm[:, h * 512:(h + 1) * 512],
                            perm_sb[:, t * 64:(t + 1) * 64],
                            OL2[:, t * 1024 + h * 512:
                                t * 1024 + (h + 1) * 512],
                            start=(t == 0), stop=(t == 2))
                rcopy(OLT[:], pm[:])

            # ========== D + T2 + E software-pipelined per w-chunk =========
            with (
                tc.tile_pool(name="ps_d", bufs=2,
                             space=bass.MemorySpace.PSUM) as pdp,
                tc.tile_pool(name="ps_e", bufs=2,
                             space=bass.MemorySpace.PSUM) as pse,
            ):
                uT2 = [None] * 4

                def emit_D(wc):
                    # u[w 128, d*128 + ab*32 + kx]; cols 64..127 of each
                    # 128-block are padding so the XBAR sends (ab, kx)
                    # to out partitions 0..63.
                    u_wc = upool.tile([128, 4096], BF, tag="u",
                                      name=f"u{wc}")
                    uv = u_wc.rearrange("p (d z kx) -> p d z kx",
                                        d=32, z=4)
                    for ab in range(2):
                        pd2 = pdp.tile([128, 1024], F32, tag="pd",
                                       name=f"pd{wc}{ab}")
                        for ns in range(2):
                            nc.tensor.matmul(
                                pd2[:, ns * 512:(ns + 1) * 512],
                                Dab_sb[:, ab * 512 + wc * 128:
                                       ab * 512 + (wc + 1) * 128],
                                OLT[:, ns * 512:(ns + 1) * 512],
                                start=True, stop=True)
                        # src cols (kx32, d32) -> dest u[d, ab, kx]
                        src = pd2.rearrange("p (kx d) -> p d kx", d=32)
                        rcopy(uv[:, :, ab, :], src)
                    return u_wc

                def emit_T2(wc, u_wc):
                    # XBAR: uT[(ab kx) (+64 garbage rows), d*128 + w]
                    t = utpool.tile([128, 4096], BF, tag="uT",
                                    name=f"uT{wc}")
                    uT2[wc] = t
                    nc.scalar.dma_start_transpose(
                        t.rearrange("p (d w) -> p d w", d=32), u_wc[:])

                def emit_E(wc):
                    for hc in range(4):
                        ob = opool.tile([128, 4096], BF, tag="osb",
                                        name=f"ob{hc}{wc}")
                        obv = ob.rearrange("p (w d) -> p d w", d=32)
                        for dgp in range(4):
                            pe2 = pse.tile([128, 1024], F32, tag="pse",
                                           name=f"pe{hc}{wc}{dgp}")
                            for hf in range(2):
                                dg = dgp * 2 + hf
                                # uT cols d*128 + w: d-group of 4 is a
                                # contiguous 512-col slice
                                nc.tensor.matmul(
                                    pe2[:, hf * 512:(hf + 1) * 512],
                                    Einv_sb[:, hc * 128:(hc + 1) * 128],
                                    uT2[wc][0:64,
                                            dg * 512:(dg + 1) * 512],
                                    start=True, stop=True)
                            # psum cols (d8, w128) -> ob[w, d] slice
                            src = pe2.rearrange("p (d w) -> p d w", w=128)
                            rcopy(obv[:, dgp * 8:(dgp + 1) * 8, :], src)
                        nc.sync.dma_start(
                            out_d[hc * 128:(hc + 1) * 128,
                                  wc * 4096:(wc + 1) * 4096],
                            ob[:])

                emit_T2(0, emit_D(0))
                u1 = emit_D(1)
                emit_T2(1, u1)
                emit_E(0)
                u2 = emit_D(2)
                emit_T2(2, u2)
                emit_E(1)
                u3 = emit_D(3)
                emit_T2(3, u3)
                emit_E(2)
                emit_E(3)
    nc.compile()
    return nc


_NC_CACHE = {}


def _get_nc():
    if "nc" not in _NC_CACHE:
        _NC_CACHE["nc"] = _build_nc()
    return _NC_CACHE["nc"]


def _wpe_from_W(W):
    # wpe[kyl*32 + c, kx*256 + kyg*32 + d] = W[kx, kyg*4 + kyl, c, d]
    Wt = np.asarray(W, dtype=np.float32).reshape(32, 8, 4, 32, 32)
    wpe = Wt.transpose(2, 3, 0, 1, 4).reshape(128, 8192)
    return np.ascontiguousarray(wpe.astype(ml_dtypes.bfloat16))


def kernel(x, W):
    xb = np.asarray(x).reshape(NCORES, H, W_ * C).astype(ml_dtypes.bfloat16)
    wpe = _wpe_from_W(W)
    nc = _get_nc()
    in_maps = [{"x": np.ascontiguousarray(xb[i]), "wpe": wpe}
               for i in range(NCORES)]
    res = run_bass_kernel_spmd(nc, in_maps, list(range(NCORES))).results
    out = np.stack([res[i]["out"].reshape(H, W_, C) for i in range(NCORES)])
    return out.astype(np.float32)


if __name__ == "__main__":
    rng = np.random.default_rng(0)
    x = rng.standard_normal((B, H, W_, C)).astype(np.float32)
    W = rng.standard_normal((MODES, MODES, C, C)).astype(np.float32) * 0.125
    out = kernel(x, W)
    print(out.shape, out.dtype)


# revision 26
# speedup vs baseline: 1.2167x; 1.2167x over previous
"""Trainium2 Bass kernel for the FNO-style FourierLayer.

  x: [8, 512, 512, 32] f32 -> rfft2 over (h, w) -> keep 32x32 modes ->
  per-mode (C x C) channel mix with W[32, 32, 32, 32] -> zero-pad -> irfft2.

Strategy: data-parallel over batch, one sample per NeuronCore (8 cores).
Only 32 of 512 frequencies survive, so instead of an FFT each core runs a
chain of small dense real matmuls against DFT basis matrices (bf16 operands,
fp32 PSUM accumulation):

  A:   P = F^T X       contract h; two w-quarters stacked on PSUM
       partitions  -> P[(q kxri) 128, (w c)] per wq-pair
  T1:  32 PE transposes of [128, 128] c-planes per pair
       -> PT[w 128, (c q kxri)]
  B:   raw = G^T PT    contract w  -> raw[ryky 64, (c rxkx)] psum accum
  Tc:  16 PE transposes of [64, (rxkx4 c32)] -> rawT[(j c) 128, (g ryky)]
       (kx = 4g' + j)
  CMB: complex re/im combine into a block-diagonal lhsT, 32 strided
       tensor_tensor ops on DVE/GpSimd:
       diag[32*kyl + c, kx*64 + kyg*8 + 2*kyl + ri] = low[kx, ky, ri, c]
  C:   per-(kx, ky-group-of-4) matmuls  OL = diag^T W  (256 matmuls of
       32 cols each; out rows (2*kyl+ri) at PSUM quadrant kyg%3)
  PRM: 3 accumulating permutation matmuls -> OLT[64 (2ky+ri), (kx d)]
  D:   u = Dab^T OLT   contract 2ky+ri -> u[w 128, (ab kx d)]
  T2:  16 PE transposes of [128, (ab kx d2)] per wc
       -> uT[(ab kx dp) 128, (t w)]   (d = 2t + dp)
  E:   out = EinvP^T uT  contract (ab kx) via parity-masked lhsT
       -> [h 128, (t4 w)] -> reorder in PSUM evac -> DMA out

All transposes are [*, 128]-column PE transposes (32KB/instr) batched 4
per PSUM tile; D/T2/E are interleaved per w-chunk so output DMA starts
early; PSUM evacuation uses [*, 1024] two-bank copies rotated across
Vector/Scalar so the PE never stalls.  Input x streams in half-tile DMA
slices; the PE warms up on a memset tile to open the p-state ramp.

DFT matrices are built on host from np.fft basis responses (this captures
the irfft Im(DC)-drop convention exactly). x, W and the matrices are cast
to bf16 on host and the output is returned as bf16 and upcast on host,
which halves DMA traffic in both directions.
"""
import numpy as np
import ml_dtypes

import concourse.bass as bass
import concourse.bacc as bacc
import concourse.mybir as mybir
from concourse import tile
from concourse.bass_utils import run_bass_kernel_spmd

B, H, W_, C = 8, 512, 512, 32
MODES = 32
N = 512
NCORES = 8

BF = mybir.dt.bfloat16
F32 = mybir.dt.float32


def _make_consts():
    h = np.arange(N)
    k = np.arange(MODES)
    ang = 2 * np.pi * np.outer(h, k) / N
    F = np.concatenate([np.cos(ang), -np.sin(ang)], axis=1)      # [512, 64]

    eye = np.eye(MODES)
    zc = np.concatenate([eye, np.zeros((MODES, N // 2 + 1 - MODES))], axis=1)
    row_re = np.fft.irfft(zc, n=N, axis=1)                        # [32, 512]
    row_im = np.fft.irfft(1j * zc, n=N, axis=1)

    # rows in interleaved (2*ky + ri) order, matching OLT rows
    Da = np.zeros((64, N))
    Db = np.zeros((64, N))
    Da[0::2] = row_re
    Da[1::2] = row_im
    Db[0::2] = row_im
    Db[1::2] = -row_re

    Einv = np.concatenate([np.cos(ang).T, np.sin(ang).T], axis=0) / N  # [64, 512]
    # parity-masked Einv for stage E: uT partitions are (ab, kx, dp)
    # with p = ab*64 + kx*2 + dp; EinvP[p, par*512 + h] selects dp == par.
    EinvP = np.zeros((128, 1024))
    for ab in range(2):
        for kx in range(32):
            for dp in range(2):
                p = ab * 64 + kx * 2 + dp
                EinvP[p, dp * 512:(dp + 1) * 512] = Einv[ab * 32 + kx]

    # F_sb[p, k*64+j] = F[k*128+p, j]
    F_sb = F.reshape(4, 128, 64).transpose(1, 0, 2).reshape(128, 256)
    Dab_sb = np.concatenate([Da, Db], axis=1)                          # [64, 1024]
    ident = np.eye(128)

    # permutation lhsT assembling OLT rows from C-stage psum quadrants:
    # psum tile t holds ky-group kyg = 3t + q at partition rows 32q + r;
    # OLT row = 8*kyg + r. Pt[32q + r, 8*(3t+q) + r] = 1.
    perm = np.zeros((3, 128, 64))
    for kyg in range(8):
        t, q = divmod(kyg, 3)
        for r in range(8):
            perm[t, 32 * q + r, 8 * kyg + r] = 1.0
    return (F_sb.astype(ml_dtypes.bfloat16), Dab_sb.astype(ml_dtypes.bfloat16),
            EinvP.astype(ml_dtypes.bfloat16), ident.astype(ml_dtypes.bfloat16),
            perm.astype(ml_dtypes.bfloat16))


def _build_nc():
    F_np, Dab_np, EinvP_np, idb_np, perm_np = _make_consts()

    nc = bacc.Bacc()
    x_d = nc.dram_tensor("x", [H, W_ * C], BF, kind="ExternalInput")
    # wpe[kyl*32 + c, kx*256 + kyg*32 + d] = W[kx, kyg*4 + kyl, c, d]
    wpe_d = nc.dram_tensor("wpe", [128, 8192], BF, kind="ExternalInput")
    out_d = nc.dram_tensor("out", [H, W_ * C], BF, kind="ExternalOutput")

    f_c = nc.inline_tensor(F_np, name="f_const")
    dab_c = nc.inline_tensor(Dab_np, name="dab_const")
    einvp_c = nc.inline_tensor(EinvP_np, name="einvp_const")
    idb_c = nc.inline_tensor(idb_np, name="idb_const")
    perm_c = nc.inline_tensor(
        np.ascontiguousarray(perm_np.transpose(1, 0, 2).reshape(128, 192)),
        name="perm_const")

    with tile.TileContext(nc) as tc:
        with (
            tc.tile_pool(name="const", bufs=1) as cpool,
            tc.tile_pool(name="xp", bufs=16) as xpool,
            tc.tile_pool(name="mid", bufs=2) as midpool,
            tc.tile_pool(name="ptp", bufs=2) as ptpool,
            tc.tile_pool(name="wp", bufs=1) as wpool,
            tc.tile_pool(name="sml", bufs=1) as smlpool,
            tc.tile_pool(name="up", bufs=2) as upool,
            tc.tile_pool(name="utp", bufs=2) as utpool,
            tc.tile_pool(name="osb", bufs=4) as opool,
        ):
            # ---- constants ----
            F_sb = cpool.tile([128, 256], BF)
            Dab_sb = cpool.tile([64, 1024], BF)
            EinvP_sb = cpool.tile([128, 1024], BF)
            ident_bf = cpool.tile([128, 128], BF)
            warm_sb = cpool.tile([128, 128], BF)
            perm_sb = cpool.tile([128, 192], BF)
            wpe_sb = wpool.tile([128, 8192], BF)

            # raw[ryky 64, rxkx*32 + c]
            raw_sb = smlpool.tile([64, 2048], BF)
            # rawT[(j c) 128, g*64 + ryky]  (kx = 4g' + j, g = rx*8 + g')
            rawT = smlpool.tile([128, 1024], BF)
            diag = smlpool.tile([128, 2048], BF)
            OL2 = smlpool.tile([128, 3072], BF)
            OLT = smlpool.tile([64, 1024], BF)

            # warm tile via memset: no DMA dependency, PE can start ~t=0
            nc.gpsimd.memset(warm_sb[:], 0.25)
            nc.gpsimd.memset(diag[:], 0.0)

            # F first (needed by first A matmul), then x streams in.
            nc.sync.dma_start(F_sb[:], f_c[:])

            # PE warmup: open the p-state ramp while first x tiles fly
            with tc.tile_pool(name="ps_w", bufs=1,
                              space=bass.MemorySpace.PSUM) as psw:
                wps = psw.tile([128, 512], F32, tag="wps", name="wps")
                for wi in range(56):
                    nc.tensor.matmul(
                        wps[:, 0:128], warm_sb[:], warm_sb[:],
                        start=True, stop=True)

            # rotating PSUM->SBUF evacuation (GPSIMD cannot access PSUM,
            # so only DVE + ACT share this work)
            _rot = [nc.vector.tensor_copy, nc.scalar.copy]
            _rix = [0]

            def rcopy(dst, src):
                fn = _rot[_rix[0] % 2]
                _rix[0] += 1
                fn(dst, src)

            # CMB ops: every 4th on GpSimd, rest on DVE
            _cix = [0]

            def ccomb(dst, a, b, op):
                fn = (nc.gpsimd.tensor_tensor if _cix[0] % 4 == 3
                      else nc.vector.tensor_tensor)
                _cix[0] += 1
                fn(dst, a, b, op)

            with (
                tc.tile_pool(name="ps_acc", bufs=4,
                             space=bass.MemorySpace.PSUM) as psa,
                tc.tile_pool(name="ps_pa", bufs=2,
                             space=bass.MemorySpace.PSUM) as ppa,
                tc.tile_pool(name="ps_pt1", bufs=2,
                             space=bass.MemorySpace.PSUM) as ppt1,
            ):
                # persistent stage-B accumulators (ns = c-octet index)
                pb = [psa.tile([64, 512], F32, tag="pb", bufs=4,
                               name=f"pb{i}") for i in range(4)]

                # ====== A + T1 + B per wq-pair, software-pipelined ======
                def emit_x_dma(pair):
                    # half-tiles [128, 2048] so pair+1's stream can start
                    # as soon as pair's first halves are consumed
                    xh = [None] * 16
                    for half in range(2):
                        for q in range(2):
                            wq = 2 * pair + q
                            for k in range(4):
                                t = xpool.tile([128, 2048], BF, tag="xk",
                                               name=f"x{wq}{k}{half}")
                                xh[q * 8 + k * 2 + half] = t
                                nc.sync.dma_start(
                                    t[:],
                                    x_d[k * 128:(k + 1) * 128,
                                        wq * 4096 + half * 2048:
                                        wq * 4096 + (half + 1) * 2048])
                    if pair == 0:
                        nc.sync.dma_start(ident_bf[:], idb_c[:])
                        nc.sync.dma_start(Dab_sb[:], dab_c[:])
                        nc.sync.dma_start(EinvP_sb[:], einvp_c[:])
                        nc.sync.dma_start(perm_sb[:], perm_c[:])
                    return xh

                def emit_A(pair, xh):
                    # P[(q kxri) 128, w*32 + c] per pair (w local to wq)
                    P_p = midpool.tile([128, 4096], BF, tag="mid",
                                       name=f"P{pair}")
                    Pv = P_p.rearrange("p (w c) -> p w c", c=32)
                    for ns in range(8):
                        half, nsl = divmod(ns, 4)
                        pa = ppa.tile([128, 512], F32, tag="pa",
                                      name=f"pa{pair}{ns}")
                        for q in range(2):
                            for k in range(4):
                                nc.tensor.matmul(
                                    pa[q * 64:(q + 1) * 64, :],
                                    F_sb[:, k * 64:(k + 1) * 64],
                                    xh[q * 8 + k * 2 + half]
                                    [:, nsl * 512:(nsl + 1) * 512],
                                    start=(k == 0), stop=(k == 3))
                        rcopy(Pv[:, ns * 16:(ns + 1) * 16, :],
                              pa.rearrange("p (w c) -> p w c", c=32))
                    return P_p

                def emit_T1B(pair, P_p):
                    # PT[w 128, q*2048 + c*64 + kxri]: q-outer so B's rhs
                    # slices are contiguous (matmul RHS needs 1 free dim)
                    PT_p = ptpool.tile([128, 4096], BF, tag="pt",
                                       name=f"PT{pair}")
                    PTq = PT_p.rearrange("p (q c k) -> p c q k", q=2, c=32)
                    Pv = P_p.rearrange("p (w c) -> p w c", c=32)
                    for cg in range(8):
                        pt1 = ppt1.tile([128, 512], BF, tag="pt1",
                                        name=f"pt1_{pair}{cg}")
                        for cl in range(4):
                            c = cg * 4 + cl
                            nc.tensor.transpose(
                                pt1[:, cl * 128:(cl + 1) * 128],
                                Pv[:, :, c], ident_bf[:])
                        rcopy(PTq[:, cg * 4:(cg + 1) * 4, :, :],
                              pt1.rearrange("p (cl q k) -> p cl q k",
                                            cl=4, q=2))
                    for q in range(2):
                        wq = 2 * pair + q
                        for ns in range(4):
                            nc.tensor.matmul(
                                pb[ns][:],
                                F_sb[:, wq * 64:(wq + 1) * 64],
                                PT_p[:, q * 2048 + ns * 512:
                                     q * 2048 + (ns + 1) * 512],
                                start=(wq == 0), stop=(wq == 3))

                xk0 = emit_x_dma(0)
                xk1 = emit_x_dma(1)
                P0 = emit_A(0, xk0)
                emit_T1B(0, P0)
                # W arrives after all x: off the phase-in critical path,
                # well before stage C needs it.
                nc.sync.dma_start(wpe_sb[:], wpe_d[:])
                P1 = emit_A(1, xk1)
                emit_T1B(1, P1)

                # pb[ns] cols (c8, rxkx) -> raw[ryky, rxkx*32 + c]
                # (rxkx-outer so Tc's transpose inputs are contiguous)
                rawv = raw_sb.rearrange("p (r c) -> p r c", c=32)
                for ns in range(4):
                    rcopy(rawv[:, :, ns * 8:(ns + 1) * 8],
                          pb[ns].rearrange("p (c k) -> p k c", c=8))

            # ====== Tc + CMB -> diag ======
            # rawT[(j c) 128, g*64 + ryky]: 16 transposes of
            # [64, 128] contiguous g-blocks batched 8 per PSUM tile
            with tc.tile_pool(name="ps_tc", bufs=2,
                              space=bass.MemorySpace.PSUM) as ptcp:
                for gh in range(2):
                    ptc = ptcp.tile([128, 512], BF, tag="tc",
                                    name=f"ptc{gh}")
                    for gl in range(8):
                        g = gh * 8 + gl
                        nc.tensor.transpose(
                            ptc[:, gl * 64:(gl + 1) * 64],
                            raw_sb[:, g * 128:(g + 1) * 128],
                            ident_bf[0:64, 0:64])
                    rcopy(rawT[:, gh * 512:(gh + 1) * 512], ptc[:])

            # CMB: rawT[(j c), (rx g' ry kyg kyl)] -> diag; kx = 4g' + j
            rTv = rawT.rearrange("p (rx gp ry kyg kyl) -> p rx gp ry kyg kyl",
                                 rx=2, gp=8, ry=2, kyg=8)
            diag_v = diag.rearrange("p (gp j kyg r) -> p gp j kyg r",
                                    gp=8, j=4, kyg=8)
            for kyl in range(4):
                prow = slice(32 * kyl, 32 * kyl + 32)
                for j in range(4):
                    srow = slice(32 * j, 32 * j + 32)
                    ccomb(diag_v[prow, :, j, :, 2 * kyl],
                          rTv[srow, 0, :, 0, :, kyl],
                          rTv[srow, 1, :, 1, :, kyl],
                          mybir.AluOpType.subtract)
                    ccomb(diag_v[prow, :, j, :, 2 * kyl + 1],
                          rTv[srow, 0, :, 1, :, kyl],
                          rTv[srow, 1, :, 0, :, kyl],
                          mybir.AluOpType.add)

            with tc.tile_pool(name="ps_c", bufs=2,
                              space=bass.MemorySpace.PSUM) as pcp:
                # C: 256 matmuls, out rows (2*kyl+ri); PSUM out partition
                # base must be 0/32/64, so 3 ky-groups per psum tile.
                # Unused psum rows are memset to 0 (the permutation matmul
                # below multiplies them by 0, and 0*NaN would poison it).
                pC = [pcp.tile([128, 1024], F32, tag="pC", bufs=3,
                               name=f"pC{i}") for i in range(3)]
                for t in range(3):
                    nc.vector.memset(pC[t][:], 0.0)

                for kx in range(32):
                    for kyg in range(8):
                        t, q = divmod(kyg, 3)
                        nc.tensor.matmul(
                            pC[t][32 * q:32 * q + 8,
                                  kx * 32:(kx + 1) * 32],
                            diag[:, kx * 64 + kyg * 8:kx * 64 + kyg * 8 + 8],
                            wpe_sb[:, kx * 256 + kyg * 32:
                                   kx * 256 + kyg * 32 + 32],
                            start=True, stop=True)
                for t in range(3):
                    rcopy(OL2[:, t * 1024:(t + 1) * 1024], pC[t][:])

            # assemble OLT rows (8*kyg + r) from the psum quadrant layout
            # with 3 accumulating permutation matmuls (disjoint out rows)
            with tc.tile_pool(name="ps_pm", bufs=1,
                              space=bass.MemorySpace.PSUM) as ppm:
                pm = ppm.tile([64, 1024], F32, tag="pm", name="pm")
                for h in range(2):
                    for t in range(3):
                        nc.tensor.matmul(
                            pm[:, h * 512:(h + 1) * 512],
                            perm_sb[:, t * 64:(t + 1) * 64],
                            OL2[:, t * 1024 + h * 512:
                                t * 1024 + (h + 1) * 512],
                            start=(t == 0), stop=(t == 2))
                rcopy(OLT[:], pm[:])

            # ========== D + T2 + E software-pipelined per w-chunk =========
            with (
                tc.tile_pool(name="ps_d", bufs=2,
                             space=bass.MemorySpace.PSUM) as pdp,
                tc.tile_pool(name="ps_t2", bufs=2,
                             space=bass.MemorySpace.PSUM) as pt2p,
                tc.tile_pool(name="ps_e", bufs=2,
                             space=bass.MemorySpace.PSUM) as pse,
            ):
                uT3 = [None] * 4

                def emit_D(wc):
                    # u[w 128, t*128 + ab*64 + kx*2 + dp]  (d = 2t + dp):
                    # t-outer so T2's transpose inputs are contiguous
                    u_wc = upool.tile([128, 2048], BF, tag="u",
                                      name=f"u{wc}")
                    uview = u_wc.rearrange("p (t ab kx dp) -> p ab kx t dp",
                                           t=16, ab=2, kx=32)
                    for ab in range(2):
                        for ns in range(2):
                            pd = pdp.tile([128, 512], F32, tag="pd",
                                          name=f"pd{wc}{ab}{ns}")
                            nc.tensor.matmul(
                                pd[:],
                                Dab_sb[:, ab * 512 + wc * 128:
                                       ab * 512 + (wc + 1) * 128],
                                OLT[:, ns * 512:(ns + 1) * 512],
                                start=True, stop=True)
                            rcopy(uview[:, ab, ns * 16:(ns + 1) * 16,
                                        :, :],
                                  pd.rearrange("p (kx t dp) -> p kx t dp",
                                               kx=16, t=16))
                    return u_wc

                def emit_T2(wc, u_wc):
                    # uT[(ab kx dp) 128, t*128 + w]  (d = 2t + dp)
                    t3 = utpool.tile([128, 2048], BF, tag="uT",
                                     name=f"uT{wc}")
                    uT3[wc] = t3
                    for tq in range(4):
                        pt2 = pt2p.tile([128, 512], BF, tag="pt2",
                                        name=f"pt2_{wc}{tq}")
                        for tl in range(4):
                            t = tq * 4 + tl
                            nc.tensor.transpose(
                                pt2[:, tl * 128:(tl + 1) * 128],
                                u_wc[:, t * 128:(t + 1) * 128],
                                ident_bf[:])
                        rcopy(t3[:, tq * 512:(tq + 1) * 512], pt2[:])

                def emit_E(wc):
                    for hc in range(4):
                        ob = opool.tile([128, 4096], BF, tag="osb",
                                        name=f"ob{hc}{wc}")
                        # ob col = w*32 + d = w*32 + t*2 + dp
                        obv = ob.rearrange("p (w t dp) -> p dp t w",
                                           t=16, dp=2)
                        for par in range(2):
                            pe2 = pse.tile([128, 1024], F32, tag="pse",
                                           name=f"pe{hc}{wc}{par}")
                            for ch in range(2):
                                nc.tensor.matmul(
                                    pe2[:, ch * 512:(ch + 1) * 512],
                                    EinvP_sb[:, par * 512 + hc * 128:
                                             par * 512 + (hc + 1) * 128],
                                    uT3[wc][:, ch * 512:(ch + 1) * 512],
                                    start=True, stop=True)
                            src = pe2.rearrange("p (t w) -> p t w", w=128)
                            rcopy(obv[:, par, 0:8, :], src)
                            pe2b = pse.tile([128, 1024], F32, tag="pse",
                                            name=f"pf{hc}{wc}{par}")
                            for ch in range(2):
                                nc.tensor.matmul(
                                    pe2b[:, ch * 512:(ch + 1) * 512],
                                    EinvP_sb[:, par * 512 + hc * 128:
                                             par * 512 + (hc + 1) * 128],
                                    uT3[wc][:, (ch + 2) * 512:
                                            (ch + 3) * 512],
                                    start=True, stop=True)
                            srcb = pe2b.rearrange("p (t w) -> p t w", w=128)
                            rcopy(obv[:, par, 8:16, :], srcb)
                        nc.sync.dma_start(
                            out_d[hc * 128:(hc + 1) * 128,
                                  wc * 4096:(wc + 1) * 4096],
                            ob[:])

                emit_T2(0, emit_D(0))
                emit_T2(1, emit_D(1))
                emit_E(0)
                emit_T2(2, emit_D(2))
                emit_E(1)
                emit_T2(3, emit_D(3))
                emit_E(2)
                emit_E(3)
    nc.compile()
    return nc


_NC_CACHE = {}


def _get_nc():
    if "nc" not in _NC_CACHE:
        _NC_CACHE["nc"] = _build_nc()
    return _NC_CACHE["nc"]


def _wpe_from_W(W):
    # wpe[kyl*32 + c, kx*256 + kyg*32 + d] = W[kx, kyg*4 + kyl, c, d]
    Wt = np.asarray(W, dtype=np.float32).reshape(32, 8, 4, 32, 32)
    wpe = Wt.transpose(2, 3, 0, 1, 4).reshape(128, 8192)
    return np.ascontiguousarray(wpe.astype(ml_dtypes.bfloat16))


def kernel(x, W):
    xb = np.asarray(x).reshape(NCORES, H, W_ * C).astype(ml_dtypes.bfloat16)
    wpe = _wpe_from_W(W)
    nc = _get_nc()
    in_maps = [{"x": np.ascontiguousarray(xb[i]), "wpe": wpe}
               for i in range(NCORES)]
    res = run_bass_kernel_spmd(nc, in_maps, list(range(NCORES))).results
    out = np.stack([res[i]["out"].reshape(H, W_, C) for i in range(NCORES)])
    return out.astype(np.float32)


if __name__ == "__main__":
    rng = np.random.default_rng(0)
    x = rng.standard_normal((B, H, W_, C)).astype(np.float32)
    W = rng.standard_normal((MODES, MODES, C, C)).astype(np.float32) * 0.125
    out = kernel(x, W)
    print(out.shape, out.dtype)


# revision 30
# speedup vs baseline: 1.9932x; 1.6382x over previous
"""Trainium2 Bass kernel for the FNO-style FourierLayer.

  x: [8, 512, 512, 32] f32 -> rfft2 over (h, w) -> keep 32x32 modes ->
  per-mode (C x C) channel mix with W[32, 32, 32, 32] -> zero-pad -> irfft2.

Strategy: data-parallel over batch, one sample per NeuronCore (8 cores).
Only 32 of 512 frequencies survive, so instead of an FFT each core runs a
chain of small dense real matmuls against DFT basis matrices (bf16 operands,
fp32 PSUM accumulation):

  A:   P = F^T X       contract h; two w-quarters stacked on PSUM
       partitions  -> P[(q kxri) 128, (w c)] per wq-pair
  T1:  32 PE transposes of [128, 128] c-planes per pair
       -> PT[w 128, (c q kxri)]
  B:   raw = G^T PT    contract w  -> raw[ryky 64, (c rxkx)] psum accum
  Tc:  16 PE transposes of [64, (rxkx4 c32)] -> rawT[(j c) 128, (g ryky)]
       (kx = 4g' + j)
  CMB: complex re/im combine into a block-diagonal lhsT, 32 strided
       tensor_tensor ops on DVE/GpSimd:
       diag[32*kyl + c, kx*64 + kyg*8 + 2*kyl + ri] = low[kx, ky, ri, c]
  C:   per-(kx, ky-group-of-4) matmuls  OL = diag^T W  (256 matmuls of
       32 cols each; out rows (2*kyl+ri) at PSUM quadrant kyg%3)
  PRM: 3 accumulating permutation matmuls -> OLT[64 (2ky+ri), (kx d)]
  D:   u = Dab^T OLT   contract 2ky+ri -> u[w 128, (ab kx d)]
  T2:  16 PE transposes of [128, (ab kx d2)] per wc
       -> uT[(ab kx dp) 128, (t w)]   (d = 2t + dp)
  E:   out = EinvP^T uT  contract (ab kx) via parity-masked lhsT
       -> [h 128, (t4 w)] -> reorder in PSUM evac -> DMA out

All transposes are [*, 128]-column PE transposes (32KB/instr) batched 4
per PSUM tile; D/T2/E are interleaved per w-chunk so output DMA starts
early; PSUM evacuation uses [*, 1024] two-bank copies rotated across
Vector/Scalar so the PE never stalls.  Input x streams in half-tile DMA
slices; the PE warms up on a memset tile to open the p-state ramp.

DFT matrices are built on host from np.fft basis responses (this captures
the irfft Im(DC)-drop convention exactly). x, W and the matrices are cast
to bf16 on host and the output is returned as bf16 and upcast on host,
which halves DMA traffic in both directions.
"""
import numpy as np
import ml_dtypes

import concourse.bass as bass
import concourse.bacc as bacc
import concourse.mybir as mybir
from concourse import tile
from concourse.bass_utils import run_bass_kernel_spmd

B, H, W_, C = 8, 512, 512, 32
MODES = 32
N = 512
NCORES = 8

BF = mybir.dt.bfloat16
F32 = mybir.dt.float32


def _make_consts():
    h = np.arange(N)
    k = np.arange(MODES)
    ang = 2 * np.pi * np.outer(h, k) / N
    F = np.concatenate([np.cos(ang), -np.sin(ang)], axis=1)      # [512, 64]

    eye = np.eye(MODES)
    zc = np.concatenate([eye, np.zeros((MODES, N // 2 + 1 - MODES))], axis=1)
    row_re = np.fft.irfft(zc, n=N, axis=1)                        # [32, 512]
    row_im = np.fft.irfft(1j * zc, n=N, axis=1)

    # rows in interleaved (2*ky + ri) order, matching OLT rows
    Da = np.zeros((64, N))
    Db = np.zeros((64, N))
    Da[0::2] = row_re
    Da[1::2] = row_im
    Db[0::2] = row_im
    Db[1::2] = -row_re

    Einv = np.concatenate([np.cos(ang).T, np.sin(ang).T], axis=0) / N  # [64, 512]
    # parity-masked Einv for stage E: uT partitions are (ab, kx, dp)
    # with p = ab*64 + kx*2 + dp; EinvP[p, par*512 + h] selects dp == par.
    EinvP = np.zeros((128, 1024))
    for ab in range(2):
        for kx in range(32):
            for dp in range(2):
                p = ab * 64 + kx * 2 + dp
                EinvP[p, dp * 512:(dp + 1) * 512] = Einv[ab * 32 + kx]

    # F_sb[p, k*64+j] = F[k*128+p, j]
    F_sb = F.reshape(4, 128, 64).transpose(1, 0, 2).reshape(128, 256)
    Dab_sb = np.concatenate([Da, Db], axis=1)                          # [64, 1024]
    ident = np.eye(128)

    # permutation lhsT assembling OLT rows from C-stage psum quadrants:
    # psum tile t holds ky-group kyg = 3t + q at partition rows 32q + r;
    # OLT row = 8*kyg + r. Pt[32q + r, 8*(3t+q) + r] = 1.
    perm = np.zeros((3, 128, 64))
    for kyg in range(8):
        t, q = divmod(kyg, 3)
        for r in range(8):
            perm[t, 32 * q + r, 8 * kyg + r] = 1.0
    return (F_sb.astype(ml_dtypes.bfloat16), Dab_sb.astype(ml_dtypes.bfloat16),
            EinvP.astype(ml_dtypes.bfloat16), ident.astype(ml_dtypes.bfloat16),
            perm.astype(ml_dtypes.bfloat16))


def _build_nc():
    F_np, Dab_np, EinvP_np, idb_np, perm_np = _make_consts()

    nc = bacc.Bacc()
    x_d = nc.dram_tensor("x", [H, W_ * C], BF, kind="ExternalInput")
    # wpe[kyl*32 + c, kx*256 + kyg*32 + d] = W[kx, kyg*4 + kyl, c, d]
    wpe_d = nc.dram_tensor("wpe", [128, 8192], BF, kind="ExternalInput")
    out_d = nc.dram_tensor("out", [H, W_ * C], BF, kind="ExternalOutput")

    f_c = nc.inline_tensor(F_np, name="f_const")
    dab_c = nc.inline_tensor(Dab_np, name="dab_const")
    einvp_c = nc.inline_tensor(EinvP_np, name="einvp_const")
    idb_c = nc.inline_tensor(idb_np, name="idb_const")
    perm_c = nc.inline_tensor(
        np.ascontiguousarray(perm_np.transpose(1, 0, 2).reshape(128, 192)),
        name="perm_const")

    with tile.TileContext(nc) as tc:
        with (
            tc.tile_pool(name="const", bufs=1) as cpool,
            tc.tile_pool(name="xp", bufs=16) as xpool,
            tc.tile_pool(name="mid", bufs=2) as midpool,
            tc.tile_pool(name="ptp", bufs=2) as ptpool,
            tc.tile_pool(name="wp", bufs=1) as wpool,
            tc.tile_pool(name="sml", bufs=1) as smlpool,
            tc.tile_pool(name="up", bufs=2) as upool,
            tc.tile_pool(name="utp", bufs=2) as utpool,
            tc.tile_pool(name="osb", bufs=4) as opool,
        ):
            # ---- constants ----
            F_sb = cpool.tile([128, 256], BF)
            Dab_sb = cpool.tile([64, 1024], BF)
            EinvP_sb = cpool.tile([128, 1024], BF)
            ident_bf = cpool.tile([128, 128], BF)
            warm_sb = cpool.tile([128, 128], BF)
            perm_sb = cpool.tile([128, 192], BF)
            wpe_sb = wpool.tile([128, 8192], BF)

            # raw[ryky 64, rxkx*32 + c]
            raw_sb = smlpool.tile([64, 2048], BF)
            # rawT[(j c) 128, g*64 + ryky]  (kx = 4g' + j, g = rx*8 + g')
            rawT = smlpool.tile([128, 1024], BF)
            diag = smlpool.tile([128, 2048], BF)
            OL2 = smlpool.tile([128, 3072], BF)
            OLT = smlpool.tile([64, 1024], BF)

            # warm tile via memset: no DMA dependency, PE can start ~t=0
            nc.gpsimd.memset(warm_sb[:], 0.25)
            nc.gpsimd.memset(diag[:], 0.0)

            # F first (needed by first A matmul), then x streams in.
            nc.sync.dma_start(F_sb[:], f_c[:])

            # PE warmup: open the p-state ramp while first x tiles fly
            with tc.tile_pool(name="ps_w", bufs=1,
                              space=bass.MemorySpace.PSUM) as psw:
                wps = psw.tile([128, 512], F32, tag="wps", name="wps")
                for wi in range(56):
                    nc.tensor.matmul(
                        wps[:, 0:128], warm_sb[:], warm_sb[:],
                        start=True, stop=True)

            # rotating PSUM->SBUF evacuation (GPSIMD cannot access PSUM,
            # so only DVE + ACT share this work)
            _rot = [nc.vector.tensor_copy, nc.scalar.copy]
            _rix = [0]

            def rcopy(dst, src):
                fn = _rot[_rix[0] % 2]
                _rix[0] += 1
                fn(dst, src)

            # CMB ops: alternate DVE / GpSimd evenly
            _cix = [0]

            def ccomb(dst, a, b, op):
                fn = (nc.gpsimd.tensor_tensor if _cix[0] % 2
                      else nc.vector.tensor_tensor)
                _cix[0] += 1
                fn(dst, a, b, op)

            with (
                tc.tile_pool(name="ps_acc", bufs=4,
                             space=bass.MemorySpace.PSUM) as psa,
                tc.tile_pool(name="ps_pa", bufs=2,
                             space=bass.MemorySpace.PSUM) as ppa,
                tc.tile_pool(name="ps_pt1", bufs=2,
                             space=bass.MemorySpace.PSUM) as ppt1,
            ):
                # persistent stage-B accumulators (ns = c-octet index)
                pb = [psa.tile([64, 512], F32, tag="pb", bufs=4,
                               name=f"pb{i}") for i in range(4)]

                # ====== A + T1 + B per wq-pair, software-pipelined ======
                def emit_x_dma(pair):
                    # half-tiles [128, 2048] so pair+1's stream can start
                    # as soon as pair's first halves are consumed
                    xh = [None] * 16
                    for half in range(2):
                        for q in range(2):
                            wq = 2 * pair + q
                            for k in range(4):
                                t = xpool.tile([128, 2048], BF, tag="xk",
                                               name=f"x{wq}{k}{half}")
                                xh[q * 8 + k * 2 + half] = t
                                nc.sync.dma_start(
                                    t[:],
                                    x_d[k * 128:(k + 1) * 128,
                                        wq * 4096 + half * 2048:
                                        wq * 4096 + (half + 1) * 2048])
                    if pair == 0:
                        nc.sync.dma_start(ident_bf[:], idb_c[:])
                        nc.sync.dma_start(Dab_sb[:], dab_c[:])
                        nc.sync.dma_start(EinvP_sb[:], einvp_c[:])
                        nc.sync.dma_start(perm_sb[:], perm_c[:])
                    return xh

                def emit_A(pair, xh):
                    # P[(q kxri) 128, w*32 + c] per pair (w local to wq)
                    P_p = midpool.tile([128, 4096], BF, tag="mid",
                                       name=f"P{pair}")
                    Pv = P_p.rearrange("p (w c) -> p w c", c=32)
                    for ns in range(8):
                        half, nsl = divmod(ns, 4)
                        pa = ppa.tile([128, 512], F32, tag="pa",
                                      name=f"pa{pair}{ns}")
                        for q in range(2):
                            for k in range(4):
                                nc.tensor.matmul(
                                    pa[q * 64:(q + 1) * 64, :],
                                    F_sb[:, k * 64:(k + 1) * 64],
                                    xh[q * 8 + k * 2 + half]
                                    [:, nsl * 512:(nsl + 1) * 512],
                                    start=(k == 0), stop=(k == 3))
                        rcopy(Pv[:, ns * 16:(ns + 1) * 16, :],
                              pa.rearrange("p (w c) -> p w c", c=32))
                    return P_p

                def emit_T1B(pair, P_p):
                    # PT[w 128, q*2048 + c*64 + kxri]: q-outer so B's rhs
                    # slices are contiguous (matmul RHS needs 1 free dim)
                    PT_p = ptpool.tile([128, 4096], BF, tag="pt",
                                       name=f"PT{pair}")
                    PTq = PT_p.rearrange("p (q c k) -> p c q k", q=2, c=32)
                    Pv = P_p.rearrange("p (w c) -> p w c", c=32)
                    for cg in range(8):
                        pt1 = ppt1.tile([128, 512], BF, tag="pt1",
                                        name=f"pt1_{pair}{cg}")
                        for cl in range(4):
                            c = cg * 4 + cl
                            nc.tensor.transpose(
                                pt1[:, cl * 128:(cl + 1) * 128],
                                Pv[:, :, c], ident_bf[:])
                        rcopy(PTq[:, cg * 4:(cg + 1) * 4, :, :],
                              pt1.rearrange("p (cl q k) -> p cl q k",
                                            cl=4, q=2))
                    for q in range(2):
                        wq = 2 * pair + q
                        for ns in range(4):
                            nc.tensor.matmul(
                                pb[ns][:],
                                F_sb[:, wq * 64:(wq + 1) * 64],
                                PT_p[:, q * 2048 + ns * 512:
                                     q * 2048 + (ns + 1) * 512],
                                start=(wq == 0), stop=(wq == 3))

                xk0 = emit_x_dma(0)
                xk1 = emit_x_dma(1)
                P0 = emit_A(0, xk0)
                emit_T1B(0, P0)
                # W arrives after all x: off the phase-in critical path,
                # well before stage C needs it.
                nc.sync.dma_start(wpe_sb[:], wpe_d[:])
                P1 = emit_A(1, xk1)
                emit_T1B(1, P1)

                # pb[ns] cols (c8, rxkx) -> raw[ryky, rxkx*32 + c]
                # (rxkx-outer so Tc's transpose inputs are contiguous)
                rawv = raw_sb.rearrange("p (r c) -> p r c", c=32)
                for ns in range(4):
                    rcopy(rawv[:, :, ns * 8:(ns + 1) * 8],
                          pb[ns].rearrange("p (c k) -> p k c", c=8))

            # ====== Tc + CMB -> diag ======
            # rawT[(j c) 128, g*64 + ryky]: 16 transposes of
            # [64, 128] contiguous g-blocks batched 8 per PSUM tile
            with tc.tile_pool(name="ps_tc", bufs=2,
                              space=bass.MemorySpace.PSUM) as ptcp:
                for gh in range(2):
                    ptc = ptcp.tile([128, 512], BF, tag="tc",
                                    name=f"ptc{gh}")
                    for gl in range(8):
                        g = gh * 8 + gl
                        nc.tensor.transpose(
                            ptc[:, gl * 64:(gl + 1) * 64],
                            raw_sb[:, g * 128:(g + 1) * 128],
                            ident_bf[0:64, 0:64])
                    rcopy(rawT[:, gh * 512:(gh + 1) * 512], ptc[:])

            # CMB: rawT[(j c), (rx g' ry kyg kyl)] -> diag; kx = 4g' + j.
            # Interleaved with C by j: C's matmuls for kx%4 == j only need
            # the 8 CMB ops of that j.
            rTv = rawT.rearrange("p (rx gp ry kyg kyl) -> p rx gp ry kyg kyl",
                                 rx=2, gp=8, ry=2, kyg=8)
            diag_v = diag.rearrange("p (gp j kyg r) -> p gp j kyg r",
                                    gp=8, j=4, kyg=8)

            with tc.tile_pool(name="ps_c", bufs=2,
                              space=bass.MemorySpace.PSUM) as pcp:
                # C: 256 matmuls, out rows (2*kyl+ri); PSUM out partition
                # base must be 0/32/64, so 3 ky-groups per psum tile.
                # Unused psum rows are memset to 0 (the permutation matmul
                # below multiplies them by 0, and 0*NaN would poison it).
                pC = [pcp.tile([128, 1024], F32, tag="pC", bufs=3,
                               name=f"pC{i}") for i in range(3)]
                for t in range(3):
                    nc.vector.memset(pC[t][:], 0.0)

                for j in range(4):
                    srow = slice(32 * j, 32 * j + 32)
                    for kyl in range(4):
                        prow = slice(32 * kyl, 32 * kyl + 32)
                        ccomb(diag_v[prow, :, j, :, 2 * kyl],
                              rTv[srow, 0, :, 0, :, kyl],
                              rTv[srow, 1, :, 1, :, kyl],
                              mybir.AluOpType.subtract)
                        ccomb(diag_v[prow, :, j, :, 2 * kyl + 1],
                              rTv[srow, 0, :, 1, :, kyl],
                              rTv[srow, 1, :, 0, :, kyl],
                              mybir.AluOpType.add)
                    for kxh in range(8):
                        kx = kxh * 4 + j
                        for kyg in range(8):
                            t, q = divmod(kyg, 3)
                            nc.tensor.matmul(
                                pC[t][32 * q:32 * q + 8,
                                      kx * 32:(kx + 1) * 32],
                                diag[:, kx * 64 + kyg * 8:
                                     kx * 64 + kyg * 8 + 8],
                                wpe_sb[:, kx * 256 + kyg * 32:
                                       kx * 256 + kyg * 32 + 32],
                                start=True, stop=True)
                for t in range(3):
                    rcopy(OL2[:, t * 1024:(t + 1) * 1024], pC[t][:])

            # assemble OLT rows (8*kyg + r) from the psum quadrant layout
            # with 3 accumulating permutation matmuls (disjoint out rows)
            with tc.tile_pool(name="ps_pm", bufs=1,
                              space=bass.MemorySpace.PSUM) as ppm:
                pm = ppm.tile([64, 1024], F32, tag="pm", name="pm")
                for h in range(2):
                    for t in range(3):
                        nc.tensor.matmul(
                            pm[:, h * 512:(h + 1) * 512],
                            perm_sb[:, t * 64:(t + 1) * 64],
                            OL2[:, t * 1024 + h * 512:
                                t * 1024 + (h + 1) * 512],
                            start=(t == 0), stop=(t == 2))
                rcopy(OLT[:], pm[:])

            # ========== D + T2 + E software-pipelined per w-chunk =========
            with (
                tc.tile_pool(name="ps_d", bufs=2,
                             space=bass.MemorySpace.PSUM) as pdp,
                tc.tile_pool(name="ps_t2", bufs=2,
                             space=bass.MemorySpace.PSUM) as pt2p,
                tc.tile_pool(name="ps_e", bufs=2,
                             space=bass.MemorySpace.PSUM) as pse,
            ):
                uT3 = [None] * 4

                def emit_D(wc):
                    # u[w 128, t*128 + ab*64 + kx*2 + dp]  (d = 2t + dp):
                    # t-outer so T2's transpose inputs are contiguous
                    u_wc = upool.tile([128, 2048], BF, tag="u",
                                      name=f"u{wc}")
                    uview = u_wc.rearrange("p (t ab kx dp) -> p ab kx t dp",
                                           t=16, ab=2, kx=32)
                    for ab in range(2):
                        for ns in range(2):
                            pd = pdp.tile([128, 512], F32, tag="pd",
                                          name=f"pd{wc}{ab}{ns}")
                            nc.tensor.matmul(
                                pd[:],
                                Dab_sb[:, ab * 512 + wc * 128:
                                       ab * 512 + (wc + 1) * 128],
                                OLT[:, ns * 512:(ns + 1) * 512],
                                start=True, stop=True)
                            rcopy(uview[:, ab, ns * 16:(ns + 1) * 16,
                                        :, :],
                                  pd.rearrange("p (kx t dp) -> p kx t dp",
                                               kx=16, t=16))
                    return u_wc

                def emit_T2(wc, u_wc):
                    # uT[(ab kx dp) 128, t*128 + w]  (d = 2t + dp)
                    t3 = utpool.tile([128, 2048], BF, tag="uT",
                                     name=f"uT{wc}")
                    uT3[wc] = t3
                    for tq in range(4):
                        pt2 = pt2p.tile([128, 512], BF, tag="pt2",
                                        name=f"pt2_{wc}{tq}")
                        for tl in range(4):
                            t = tq * 4 + tl
                            nc.tensor.transpose(
                                pt2[:, tl * 128:(tl + 1) * 128],
                                u_wc[:, t * 128:(t + 1) * 128],
                                ident_bf[:])
                        rcopy(t3[:, tq * 512:(tq + 1) * 512], pt2[:])

                def emit_E(wc):
                    for hc in range(4):
                        # ob col = d*128 + w (d = 2t + dp): evac copies
                        # stay contiguous; the host un-permutes (d, w)
                        ob = opool.tile([128, 4096], BF, tag="osb",
                                        name=f"ob{hc}{wc}")
                        obv = ob.rearrange("p (t dp w) -> p dp t w",
                                           t=16, dp=2)
                        for par in range(2):
                            pe2 = pse.tile([128, 1024], F32, tag="pse",
                                           name=f"pe{hc}{wc}{par}")
                            for ch in range(2):
                                nc.tensor.matmul(
                                    pe2[:, ch * 512:(ch + 1) * 512],
                                    EinvP_sb[:, par * 512 + hc * 128:
                                             par * 512 + (hc + 1) * 128],
                                    uT3[wc][:, ch * 512:(ch + 1) * 512],
                                    start=True, stop=True)
                            src = pe2.rearrange("p (t w) -> p t w", w=128)
                            rcopy(obv[:, par, 0:8, :], src)
                            pe2b = pse.tile([128, 1024], F32, tag="pse",
                                            name=f"pf{hc}{wc}{par}")
                            for ch in range(2):
                                nc.tensor.matmul(
                                    pe2b[:, ch * 512:(ch + 1) * 512],
                                    EinvP_sb[:, par * 512 + hc * 128:
                                             par * 512 + (hc + 1) * 128],
                                    uT3[wc][:, (ch + 2) * 512:
                                            (ch + 3) * 512],
                                    start=True, stop=True)
                            srcb = pe2b.rearrange("p (t w) -> p t w", w=128)
                            rcopy(obv[:, par, 8:16, :], srcb)
                        nc.sync.dma_start(
                            out_d[hc * 128:(hc + 1) * 128,
                                  wc * 4096:(wc + 1) * 4096],
                            ob[:])

                emit_T2(0, emit_D(0))
                emit_T2(1, emit_D(1))
                emit_E(0)
                emit_T2(2, emit_D(2))
                emit_E(1)
                emit_T2(3, emit_D(3))
                emit_E(2)
                emit_E(3)
    nc.compile()
    return nc


_NC_CACHE = {}


def _get_nc():
    if "nc" not in _NC_CACHE:
        _NC_CACHE["nc"] = _build_nc()
    return _NC_CACHE["nc"]


def _wpe_from_W(W):
    # wpe[kyl*32 + c, kx*256 + kyg*32 + d] = W[kx, kyg*4 + kyl, c, d]
    Wt = np.asarray(W, dtype=np.float32).reshape(32, 8, 4, 32, 32)
    wpe = Wt.transpose(2, 3, 0, 1, 4).reshape(128, 8192)
    return np.ascontiguousarray(wpe.astype(ml_dtypes.bfloat16))


def kernel(x, W):
    xb = np.asarray(x).reshape(NCORES, H, W_ * C).astype(ml_dtypes.bfloat16)
    wpe = _wpe_from_W(W)
    nc = _get_nc()
    in_maps = [{"x": np.ascontiguousarray(xb[i]), "wpe": wpe}
               for i in range(NCORES)]
    res = run_bass_kernel_spmd(nc, in_maps, list(range(NCORES))).results
    # device emits [h, wc, d, w]; un-permute (d, w) -> (w, d) on host
    out = np.stack([res[i]["out"].reshape(H, 4, C, 128)
                    for i in range(NCORES)])
    out = out.astype(np.float32).transpose(0, 1, 2, 4, 3)
    return np.ascontiguousarray(out).reshape(B, H, W_, C)


if __name__ == "__main__":
    rng = np.random.default_rng(0)
    x = rng.standard_normal((B, H, W_, C)).astype(np.float32)
    W = rng.standard_normal((MODES, MODES, C, C)).astype(np.float32) * 0.125
    out = kernel(x, W)
    print(out.shape, out.dtype)
